# revision 1
# baseline (speedup 1.0000x reference)
"""Distributed Trainium2 kernel for the dense-graph GNN layer.

Math: with xn = x/||x|| (rows), G = xn@xn.T, d = rsqrt(G@1),
out = (diag(d) G diag(d) x) W.  The N x N Gram matrix is never needed:
  G @ 1        = xn @ t,            t = colsum(xn)            [D]
  diag(d) G diag(d) x = f * (x @ z),  z = x.T @ diag(f) @ x   [D, D]
  f_i = d_i / ||x_i||   (combines both scalings; z is symmetric)
  out = f * (x @ (z @ W))
So each core processes its 1024-row shard with O(N D^2) flops and the only
cross-core traffic is an AllGather of a [1,256] colsum partial and an
AllReduce of the [256,256] (z @ W) partial.
"""

import os
import sys

import numpy as np

for _p in ("/opt/trn_rl_repo", "/root/.axon_site/_ro/trn_rl_repo"):
    if os.path.isdir(_p) and _p not in sys.path:
        sys.path.insert(0, _p)

import concourse.bacc as bacc
import concourse.mybir as mybir
import concourse.tile as tile
import concourse.masks as masks
from concourse import bass_utils

R = 8                 # cores
N, D = 8192, 256
NL = N // R           # 1024 rows per core
P = 128
T = NL // P           # 8 row tiles per core
F32 = mybir.dt.float32
BF16 = mybir.dt.bfloat16
AF = mybir.ActivationFunctionType
ALU = mybir.AluOpType

_cache = {}


def _program(tc, x, W, out):
    nc = tc.nc
    rg = [list(range(R))]
    if True:
        with (
            tc.tile_pool(name="persist", bufs=1) as pp,
            tc.tile_pool(name="work", bufs=3) as wp,
            tc.tile_pool(name="psum", bufs=1, space="PSUM") as psp,
            tc.tile_pool(name="psumw", bufs=4, space="PSUM") as psw,
            tc.tile_pool(name="dram", bufs=1, space="DRAM") as dp,
        ):
            x_all = pp.tile([P, T * D], F32)      # row tile i at [:, i*D:(i+1)*D]
            xb_all = pp.tile([P, T * D], BF16)    # bf16 copy of x
            g_all = pp.tile([P, T * D], BF16)     # f * x (bf16)
            xT_all = pp.tile([P, 2 * NL], BF16)   # x.T chunk c at [:, c*NL + i*P]
            W_sb = pp.tile([P, 2 * D], F32)       # W k-chunk kc at [:, kc*D]
            Wb_sb = pp.tile([P, 2 * D], BF16)
            zw_sb = pp.tile([P, 2 * D], BF16)     # zw a-chunk ka at [:, ka*D]
            zT_sb = pp.tile([P, 2 * D], BF16)

            ss = pp.tile([P, T], F32)
            invn = pp.tile([P, T], F32)
            nrm = pp.tile([P, T], F32)
            stl = pp.tile([P, T], F32)
            s_t = pp.tile([P, T], F32)
            sq_s = pp.tile([P, T], F32)
            dd = pp.tile([P, T], F32)
            f_t = pp.tile([P, T], F32)

            ident = pp.tile([P, P], F32)
            masks.make_identity(nc, ident[:])
            ones8 = pp.tile([8, P], F32)
            nc.gpsimd.memset(ones8[:], 1.0)

            cc_t_in = dp.tile([1, D], F32)
            cc_t_out = dp.tile([R, D], F32)
            cc_zw_in = dp.tile([2 * P, D], BF16)
            cc_zw_out = dp.tile([2 * P, D], BF16)

            for kc in range(2):
                nc.sync.dma_start(W_sb[:, kc * D:(kc + 1) * D], W[kc * P:(kc + 1) * P, :])
            nc.vector.tensor_copy(Wb_sb[:], W_sb[:])

            # ---- phase A: load shard, row norms, colsum(xn) partial ----
            for i in range(T):
                xs = x_all[:, i * D:(i + 1) * D]
                nc.sync.dma_start(xs, x[i * P:(i + 1) * P, :])
                scr = wp.tile([P, D], F32, tag="scr", name=f"scr{i}")
                nc.scalar.activation(scr[:], xs, AF.Square, accum_out=ss[:, i:i + 1])
                nc.vector.tensor_copy(xb_all[:, i * D:(i + 1) * D], xs)
            nc.scalar.activation(nrm[:], ss[:], AF.Sqrt)
            nc.vector.reciprocal(invn[:], nrm[:])

            psum_t = psp.tile([1, D], F32, name="psum_t")
            for i in range(T):
                nc.tensor.matmul(
                    psum_t[:], lhsT=invn[:, i:i + 1], rhs=x_all[:, i * D:(i + 1) * D],
                    start=(i == 0), stop=(i == T - 1),
                )
            t_sb = pp.tile([1, D], F32)
            nc.vector.tensor_copy(t_sb[:], psum_t[:])
            nc.sync.dma_start(cc_t_in[:], t_sb[:])
            nc.gpsimd.collective_compute(
                "AllGather", ALU.bypass, replica_groups=rg,
                ins=[cc_t_in.opt()], outs=[cc_t_out.opt()],
            )

            # x.T via PE transposes (independent of the collective -> overlaps it)
            for i in range(T):
                for c in range(2):
                    pt = psw.tile([P, P], F32, tag="pw", name=f"pt{i}_{c}")
                    nc.tensor.transpose(
                        pt[:], x_all[:, i * D + c * P: i * D + (c + 1) * P], ident[:]
                    )
                    nc.vector.tensor_copy(xT_all[:, c * NL + i * P: c * NL + (i + 1) * P], pt[:])

            tg_sb = pp.tile([8, D], F32)
            nc.sync.dma_start(tg_sb[:], cc_t_out[:])
            # sum the 8 rank partials AND broadcast to 128 partitions in one matmul
            psum_tb = psp.tile([P, D], F32, name="psum_tb")
            nc.tensor.matmul(psum_tb[:], lhsT=ones8[:], rhs=tg_sb[:], start=True, stop=True)

            # ---- phase B: degrees, f, g = f*x, zT partial, zw partial ----
            tb_sb = pp.tile([P, D], F32)
            nc.vector.tensor_copy(tb_sb[:], psum_tb[:])
            big_scr = pp.tile([P, T * D], F32)
            t_ap = tb_sb[:]
            from concourse.bass_types import AP as _AP
            t_rep = _AP(t_ap.tensor, t_ap.offset, [t_ap.ap[0], [0, T], t_ap.ap[1]])
            x3 = x_all[:].rearrange("p (t d) -> p t d", t=T)
            s3 = big_scr[:].rearrange("p (t d) -> p t d", t=T)
            nc.vector.tensor_mul(s3, x3, t_rep)
            nc.vector.tensor_reduce(stl[:], s3, axis=mybir.AxisListType.X, op=ALU.add)
            nc.vector.tensor_mul(s_t[:], stl[:], invn[:])       # s = rowsum * invn
            nc.scalar.activation(sq_s[:], s_t[:], AF.Sqrt)
            nc.vector.reciprocal(dd[:], sq_s[:])                # d = rsqrt(s)
            nc.vector.tensor_mul(f_t[:], dd[:], invn[:])        # f = d * invn
            for i in range(T):
                nc.scalar.mul(g_all[:, i * D:(i + 1) * D], x_all[:, i * D:(i + 1) * D],
                              f_t[:, i:i + 1])

            psum_zT0 = psp.tile([P, D], F32, name="pzT0")
            psum_zT1 = psp.tile([P, D], F32, name="pzT1")
            for i in range(T):
                for c, pz in ((0, psum_zT0), (1, psum_zT1)):
                    nc.tensor.matmul(
                        pz[:], lhsT=xb_all[:, i * D + c * P: i * D + (c + 1) * P],
                        rhs=g_all[:, i * D:(i + 1) * D],
                        start=(i == 0), stop=(i == T - 1),
                    )
            for c, pz in ((0, psum_zT0), (1, psum_zT1)):
                nc.vector.tensor_copy(zT_sb[:, c * D:(c + 1) * D], pz[:])


            # zw partial = z_p @ W (fold the W GEMM before the collective)
            for m in range(2):
                pzw = psw.tile([P, D], F32, tag="pw", name=f"pzw{m}")
                for kc in range(2):
                    nc.tensor.matmul(
                        pzw[:], lhsT=zT_sb[:, kc * D + m * P: kc * D + (m + 1) * P],
                        rhs=Wb_sb[:, kc * D:(kc + 1) * D],
                        start=(kc == 0), stop=(kc == 1),
                    )
                zwp_sb = wp.tile([P, D], BF16, tag="zwp", name=f"zwp{m}")
                nc.vector.tensor_copy(zwp_sb[:], pzw[:])
                nc.sync.dma_start(cc_zw_in[m * P:(m + 1) * P, :], zwp_sb[:])
            nc.gpsimd.collective_compute(
                "AllReduce", ALU.add, replica_groups=rg,
                ins=[cc_zw_in.opt()], outs=[cc_zw_out.opt()],
            )
            for ka in range(2):
                nc.sync.dma_start(zw_sb[:, ka * D:(ka + 1) * D], cc_zw_out[ka * P:(ka + 1) * P, :])

            # ---- phase C: out = f * (x @ zw) ----
            for i in range(T):
                po = psw.tile([P, D], F32, tag="pw", name=f"po{i}")
                for ka in range(2):
                    nc.tensor.matmul(
                        po[:], lhsT=xT_all[:, ka * NL + i * P: ka * NL + (i + 1) * P],
                        rhs=zw_sb[:, ka * D:(ka + 1) * D],
                        start=(ka == 0), stop=(ka == 1),
                    )
                o_sb = wp.tile([P, D], F32, tag="osb", name=f"osb{i}")
                nc.scalar.mul(o_sb[:], po[:], f_t[:, i:i + 1])
                nc.sync.dma_start(out[i * P:(i + 1) * P, :], o_sb[:])


def _build():
    nc = bacc.Bacc("TRN2", target_bir_lowering=False, debug=False, num_devices=R)
    x = nc.dram_tensor("x", [NL, D], F32, kind="ExternalInput")
    W = nc.dram_tensor("W", [D, D], F32, kind="ExternalInput")
    out = nc.dram_tensor("out", [NL, D], F32, kind="ExternalOutput")
    with tile.TileContext(nc) as tc:
        _program(tc, x.ap() if hasattr(x, "ap") else x, W.ap() if hasattr(W, "ap") else W, out.ap() if hasattr(out, "ap") else out)
    nc.finalize()
    return nc


def _run(inputs, trace=False):
    if "nc" not in _cache:
        _cache["nc"] = _build()
    nc = _cache["nc"]
    x = np.ascontiguousarray(inputs["x"], dtype=np.float32)
    W = np.ascontiguousarray(inputs["W"], dtype=np.float32)
    in_maps = [{"x": x[r * NL:(r + 1) * NL], "W": W} for r in range(R)]
    res = bass_utils.run_bass_kernel_spmd(
        nc, in_maps, core_ids=list(range(R)), trace=trace,
    )
    out = np.concatenate([res.results[r]["out"] for r in range(R)], axis=0)
    return out, res


def kernel(**inputs) -> np.ndarray:
    out, _ = _run(inputs, trace=False)
    return out



# revision 3
# speedup vs baseline: 1.1838x; 1.1838x over previous
"""Distributed Trainium2 kernel for the dense-graph GNN layer.

Math: with xn = x/||x|| (rows), G = xn@xn.T, d = rsqrt(G@1),
out = (diag(d) G diag(d) x) W.  The N x N Gram matrix is never needed:
  t = colsum(xn) = X^T invn                                   [D]
  r_i = x_i . t ;  f_i = rsqrt(||x_i|| * r_i)
  z = X^T diag(f) X                                           [D, D]
  out = f_loc * (X_loc @ (z @ W))

Distribution: collectives in this environment cost ~40us+ each (measured:
both AllGather and AllReduce stall every core for tens of us in the
axon/fake_nrt runtime), so this kernel uses ZERO collectives.  Every core
receives the FULL x (rolled so its own 1024 rows come first - the global
reductions t and z are permutation-invariant) and redundantly computes the
global reductions, then produces only its local 1024-row output slice.
Per-core cost is dominated by streaming the 8MB x from HBM once.
"""

import os
import sys

import numpy as np

for _p in ("/opt/trn_rl_repo", "/root/.axon_site/_ro/trn_rl_repo"):
    if os.path.isdir(_p) and _p not in sys.path:
        sys.path.insert(0, _p)

import concourse.bacc as bacc
import concourse.mybir as mybir
import concourse.tile as tile
import concourse.masks as masks
from concourse import bass_utils
from concourse.bass_types import AP as _AP

R = 8                  # cores
N, D = 8192, 256
NL = N // R            # 1024 rows per core (local shard)
P = 128
T = N // P             # 64 row tiles per core (full x)
TL = NL // P           # 8 local row tiles
CH = 8                 # tiles per DMA chunk
NCH = T // CH          # 8 chunks
CW = CH * D            # chunk width in columns (2048)
F32 = mybir.dt.float32
BF16 = mybir.dt.bfloat16
AF = mybir.ActivationFunctionType
ALU = mybir.AluOpType
AX = mybir.AxisListType

_cache = {}


def _bcast_free(ap, n, pos=1):
    """Insert a stride-0 dim of size n at free position `pos`."""
    dims = list(ap.ap)
    dims.insert(pos, [0, n])
    return _AP(ap.tensor, ap.offset, dims)


def _program(tc, x, W, out):
    nc = tc.nc
    with (
        tc.tile_pool(name="persist", bufs=1) as pp,
        tc.tile_pool(name="work", bufs=3) as wp,
        tc.tile_pool(name="psA", bufs=1, space="PSUM") as psA,
        tc.tile_pool(name="psW", bufs=4, space="PSUM") as psW,
    ):
        xb_all = pp.tile([P, T * D], BF16)       # bf16 x, resident (4MB)
        nsq = pp.tile([P, T], BF16)              # row sum-of-squares
        nrm = pp.tile([P, T], F32)               # ||x_i||
        invn = pp.tile([P, T], F32)
        invn_bf = pp.tile([P, T], BF16)
        r_bf = pp.tile([P, T], BF16)             # x_i . t
        p_t = pp.tile([P, T], F32)
        sp_t = pp.tile([P, T], F32)
        f_t = pp.tile([P, T], F32)               # f = rsqrt(nrm * r)

        W_sb = pp.tile([P, 2 * D], F32)          # W k-chunk kc at [:, kc*D]
        Wb_sb = pp.tile([P, 2 * D], BF16)
        t_sb = pp.tile([1, D], BF16)
        ones_bf = pp.tile([1, P], BF16)
        tb_sb = pp.tile([P, D], BF16)            # t broadcast to 128 partitions
        ident_f = pp.tile([P, P], F32)
        ident_bf = pp.tile([P, P], BF16)
        xbT = pp.tile([P, 2 * NL], BF16)         # local x^T: block (i,c) at (2i+c)*P
        z_top_sb = pp.tile([P, D], BF16)         # [z11 | z12]
        z22_sb = pp.tile([P, P], BF16)
        z21_sb = pp.tile([P, P], BF16)
        zw_sb = pp.tile([P, 2 * D], BF16)        # zw rows 0:128 at [:,0:D], 128:256 at [:,D:2D]

        t_ps = psA.tile([1, D], F32, name="t_ps")
        tb_ps = psA.tile([P, D], F32, name="tb_ps")
        z_top_ps = psA.tile([P, D], F32, name="z_top_ps")
        z22_ps = psA.tile([P, P], F32, name="z22_ps")

        masks.make_identity(nc, ident_f[:])
        nc.vector.tensor_copy(ident_bf[:], ident_f[:])
        nc.gpsimd.memset(ones_bf[:], 1.0)

        for kc in range(2):
            nc.sync.dma_start(W_sb[:, kc * D:(kc + 1) * D], W[kc * P:(kc + 1) * P, :])
        nc.vector.tensor_copy(Wb_sb[:], W_sb[:])

        # ---- pass 1 (overlapped with DMA): cast, row sumsq, t accumulation ----
        x_chs = []
        for c in range(NCH):
            x_ch = wp.tile([P, CW], F32, tag="xch", name=f"xch{c}")
            src = x[c * CH * P:(c + 1) * CH * P, :].rearrange(
                "(t p) d -> p t d", p=P
            )
            nc.sync.dma_start(x_ch[:].rearrange("p (t d) -> p t d", t=CH), src)
            x_chs.append(x_ch)

        for c in range(NCH):
            x_ch = x_chs[c]
            cs = slice(c * CH, (c + 1) * CH)
            xb_ch = xb_all[:, c * CW:(c + 1) * CW]
            nc.vector.tensor_copy(xb_ch, x_ch[:])
            sq_ch = wp.tile([P, CW], BF16, tag="sq", name=f"sq{c}")
            nc.scalar.activation(sq_ch[:], x_ch[:], AF.Square)
            nc.vector.tensor_reduce(
                nsq[:, cs], sq_ch[:].rearrange("p (t d) -> p t d", t=CH),
                axis=AX.X, op=ALU.add,
            )
            nc.scalar.activation(nrm[:, cs], nsq[:, cs], AF.Sqrt)
            nc.vector.reciprocal(invn[:, cs], nrm[:, cs])
            nc.vector.tensor_copy(invn_bf[:, cs], invn[:, cs])

            if c == 0:
                # local x^T for the final GEMM - PE is idle during the load
                for i in range(TL):
                    for h in range(2):
                        pt = psW.tile([P, P], BF16, tag="pw", name=f"pt{i}_{h}")
                        nc.tensor.transpose(
                            pt[:], xb_all[:, i * D + h * P:i * D + (h + 1) * P],
                            ident_bf[:],
                        )
                        nc.vector.tensor_copy(
                            xbT[:, (2 * i + h) * P:(2 * i + h + 1) * P], pt[:]
                        )

            for i in range(c * CH, (c + 1) * CH):
                nc.tensor.matmul(
                    t_ps[:], lhsT=invn_bf[:, i:i + 1],
                    rhs=xb_all[:, i * D:(i + 1) * D],
                    start=(i == 0), stop=(i == T - 1),
                )

        # ---- barrier: t ready; broadcast to 128 partitions ----
        nc.vector.tensor_copy(t_sb[:], t_ps[:])
        nc.tensor.matmul(tb_ps[:], lhsT=ones_bf[:], rhs=t_sb[:], start=True, stop=True)
        nc.vector.tensor_copy(tb_sb[:], tb_ps[:])

        # ---- pass 2: r = x.t, f, g = f*x, z accumulation ----
        tb3 = _bcast_free(tb_sb[:], CH)          # [128, CH, 256] stride-0
        for c in range(NCH):
            cs = slice(c * CH, (c + 1) * CH)
            xb3 = xb_all[:, c * CW:(c + 1) * CW].rearrange("p (t d) -> p t d", t=CH)
            u_ch = wp.tile([P, CW], BF16, tag="u", name=f"u{c}")
            u3 = u_ch[:].rearrange("p (t d) -> p t d", t=CH)
            nc.vector.tensor_mul(u3, xb3, tb3)
            nc.vector.tensor_reduce(r_bf[:, cs], u3, axis=AX.X, op=ALU.add)
            nc.vector.tensor_mul(p_t[:, cs], nrm[:, cs], r_bf[:, cs])
            nc.scalar.activation(sp_t[:, cs], p_t[:, cs], AF.Sqrt)
            nc.vector.reciprocal(f_t[:, cs], sp_t[:, cs])

            g_ch = wp.tile([P, CW], BF16, tag="g", name=f"g{c}")
            for j in range(CH):
                i = c * CH + j
                g_i = g_ch[:, j * D:(j + 1) * D]
                nc.vector.tensor_scalar_mul(g_i, xb_all[:, i * D:(i + 1) * D],
                                            f_t[:, i:i + 1])
                nc.tensor.matmul(
                    z_top_ps[:], lhsT=xb_all[:, i * D:i * D + P], rhs=g_i,
                    start=(i == 0), stop=(i == T - 1),
                )
                nc.tensor.matmul(
                    z22_ps[:], lhsT=xb_all[:, i * D + P:(i + 1) * D],
                    rhs=g_ch[:, j * D + P:(j + 1) * D],
                    start=(i == 0), stop=(i == T - 1),
                )

        # ---- zw = z @ W using symmetry (z21 = z12^T) ----
        nc.vector.tensor_copy(z_top_sb[:], z_top_ps[:])
        nc.vector.tensor_copy(z22_sb[:], z22_ps[:])
        zT_ps = psW.tile([P, P], BF16, tag="pw", name="zT")
        nc.tensor.transpose(zT_ps[:], z_top_sb[:, P:D], ident_bf[:])
        nc.vector.tensor_copy(z21_sb[:], zT_ps[:])

        for half, (lhs1, lhs2) in enumerate(
            ((z_top_sb[:, 0:P], z21_sb[:]),        # zw_top = z11 W1 + (z12^T)^T W2
             (z_top_sb[:, P:D], z22_sb[:]))        # zw_bot = z12^T W1 + z22 W2
        ):
            zw_ps = psW.tile([P, D], F32, tag="pw", name=f"zw{half}")
            nc.tensor.matmul(zw_ps[:], lhsT=lhs1, rhs=Wb_sb[:, 0:D],
                             start=True, stop=False)
            nc.tensor.matmul(zw_ps[:], lhsT=lhs2, rhs=Wb_sb[:, D:2 * D],
                             start=False, stop=True)
            nc.vector.tensor_copy(zw_sb[:, half * D:(half + 1) * D], zw_ps[:])

        # ---- final: out_i = f_i * (x_i @ zw) for the 8 local tiles ----
        for i in range(TL):
            o_ps = psW.tile([P, D], F32, tag="pw", name=f"o{i}")
            for h in range(2):
                nc.tensor.matmul(
                    o_ps[:], lhsT=xbT[:, (2 * i + h) * P:(2 * i + h + 1) * P],
                    rhs=zw_sb[:, h * D:(h + 1) * D],
                    start=(h == 0), stop=(h == 1),
                )
            o_sb = wp.tile([P, D], F32, tag="osb", name=f"osb{i}")
            nc.scalar.mul(o_sb[:], o_ps[:], f_t[:, i:i + 1])
            nc.sync.dma_start(out[i * P:(i + 1) * P, :], o_sb[:])


def _build():
    nc = bacc.Bacc("TRN2", target_bir_lowering=False, debug=False, num_devices=R)
    x = nc.dram_tensor("x", [N, D], F32, kind="ExternalInput")
    W = nc.dram_tensor("W", [D, D], F32, kind="ExternalInput")
    out = nc.dram_tensor("out", [NL, D], F32, kind="ExternalOutput")
    with nc.allow_low_precision("bf16 row reductions; validated ~1.6e-3 rel err"):
        with tile.TileContext(nc) as tc:
            _program(
                tc,
                x.ap() if hasattr(x, "ap") else x,
                W.ap() if hasattr(W, "ap") else W,
                out.ap() if hasattr(out, "ap") else out,
            )
    nc.finalize()
    return nc


def _run(inputs, trace=False):
    if "nc" not in _cache:
        _cache["nc"] = _build()
    nc = _cache["nc"]
    x = np.ascontiguousarray(inputs["x"], dtype=np.float32)
    W = np.ascontiguousarray(inputs["W"], dtype=np.float32)
    in_maps = [
        {"x": np.roll(x, -r * NL, axis=0), "W": W} for r in range(R)
    ]
    res = bass_utils.run_bass_kernel_spmd(
        nc, in_maps, core_ids=list(range(R)), trace=trace,
    )
    out = np.concatenate([res.results[r]["out"] for r in range(R)], axis=0)
    return out, res


def kernel(**inputs) -> np.ndarray:
    out, _ = _run(inputs, trace=False)
    return out


# revision 4
# speedup vs baseline: 1.2839x; 1.0846x over previous
"""Distributed Trainium2 kernel for the dense-graph GNN layer.

Math: with xn = x/||x|| (rows), G = xn@xn.T, d = rsqrt(G@1),
out = (diag(d) G diag(d) x) W.  The N x N Gram matrix is never needed:
  t = colsum(xn) = X^T invn                                   [D]
  r_i = x_i . t ;  f_i = rsqrt(||x_i|| * r_i)
  z = X^T diag(f) X                                           [D, D]
  out = f_loc * (X_loc @ (z @ W))

Distribution: collectives in this environment cost ~40us+ each (measured:
both AllGather and AllReduce stall every core for tens of us in the
axon/fake_nrt runtime), so this kernel uses ZERO collectives.  Every core
receives the FULL x (rolled so its own 1024 rows come first - the global
reductions t and z are permutation-invariant) and redundantly computes the
global reductions, then produces only its local 1024-row output slice.
Per-core cost is dominated by streaming the 8MB x from HBM once.

Row layout: within each 1024-row chunk, partition p holds rows 8p..8p+7
(8KB contiguous per partition per chunk -> efficient DMA descriptors).
All global reductions are row-permutation-invariant; the local output
store inverts the same mapping.
"""

import os
import sys

import numpy as np

for _p in ("/opt/trn_rl_repo", "/root/.axon_site/_ro/trn_rl_repo"):
    if os.path.isdir(_p) and _p not in sys.path:
        sys.path.insert(0, _p)

import concourse.bacc as bacc
import concourse.mybir as mybir
import concourse.tile as tile
import concourse.masks as masks
from concourse import bass_utils
from concourse.bass_types import AP as _AP

R = 8                  # cores
N, D = 8192, 256
NL = N // R            # 1024 rows per core (local shard)
P = 128
T = N // P             # 64 row tiles per core (full x)
TL = NL // P           # 8 local row tiles
CH = 8                 # tiles per DMA chunk
NCH = T // CH          # 8 chunks
CW = CH * D            # chunk width in columns (2048)
F32 = mybir.dt.float32
BF16 = mybir.dt.bfloat16
AF = mybir.ActivationFunctionType
ALU = mybir.AluOpType
AX = mybir.AxisListType

_cache = {}


def _bcast_free(ap, n, pos=1):
    """Insert a stride-0 dim of size n at free position `pos`."""
    dims = list(ap.ap)
    dims.insert(pos, [0, n])
    return _AP(ap.tensor, ap.offset, dims)


def _rowsum(nc, out_col, ch3, scratch3):
    """Row-sum of ch3 [128, CH, 256] -> out_col [128, CH].
    Three 2x halving adds into scratch, then a 1x reduce of [128, CH, 32]."""
    w = 128
    nc.vector.tensor_add(scratch3[:, :, 0:w], ch3[:, :, 0:w], ch3[:, :, w:2 * w])
    while w > 32:
        h = w // 2
        nc.vector.tensor_add(scratch3[:, :, 0:h], scratch3[:, :, 0:h],
                             scratch3[:, :, h:2 * h])
        w = h
    nc.vector.tensor_reduce(out_col, scratch3[:, :, 0:w], axis=AX.X, op=ALU.add)


def _program(tc, x, W, out):
    nc = tc.nc
    with (
        tc.tile_pool(name="persist", bufs=1) as pp,
        tc.tile_pool(name="work", bufs=3) as wp,
        tc.tile_pool(name="psA", bufs=1, space="PSUM") as psA,
        tc.tile_pool(name="psW", bufs=4, space="PSUM") as psW,
    ):
        xb_all = pp.tile([P, T * D], BF16)       # bf16 x, resident (4MB)
        nsq = pp.tile([P, T], BF16)              # row sum-of-squares
        nrm = pp.tile([P, T], F32)               # ||x_i||
        invn = pp.tile([P, T], F32)
        invn_bf = pp.tile([P, T], BF16)
        r_bf = pp.tile([P, T], BF16)             # x_i . t
        p_t = pp.tile([P, T], F32)
        sp_t = pp.tile([P, T], F32)
        f_t = pp.tile([P, T], F32)               # f = rsqrt(nrm * r)

        W_sb = pp.tile([P, 2 * D], F32)          # W k-chunk kc at [:, kc*D]
        Wb_sb = pp.tile([P, 2 * D], BF16)
        t_sb = pp.tile([1, D], BF16)
        ones_bf = pp.tile([1, P], BF16)
        tb_sb = pp.tile([P, D], BF16)            # t broadcast to 128 partitions
        ident_f = pp.tile([P, P], F32)
        ident_bf = pp.tile([P, P], BF16)
        xbT = pp.tile([P, 2 * NL], BF16)         # local x^T: block (i,c) at (2i+c)*P
        z_top_sb = pp.tile([P, D], BF16)         # [z11 | z12]
        z22_sb = pp.tile([P, P], BF16)
        z21_sb = pp.tile([P, P], BF16)
        zw_sb = pp.tile([P, 2 * D], BF16)        # zw rows 0:128 at [:,0:D], 128:256 at [:,D:2D]

        t_ps = psA.tile([1, D], F32, name="t_ps")
        tb_ps = psA.tile([P, D], F32, name="tb_ps")
        z_top_ps = psA.tile([P, D], F32, name="z_top_ps")
        z22_ps = psA.tile([P, P], F32, name="z22_ps")

        # x chunk DMAs first - everything else hides under them
        x_chs = []
        for c in range(NCH):
            x_ch = wp.tile([P, CW], F32, tag="xch", name=f"xch{c}")
            src = x[c * CH * P:(c + 1) * CH * P, :].rearrange(
                "(p j) d -> p j d", p=P
            )
            nc.sync.dma_start(x_ch[:].rearrange("p (j d) -> p j d", j=CH), src)
            x_chs.append(x_ch)

        for kc in range(2):
            nc.sync.dma_start(W_sb[:, kc * D:(kc + 1) * D], W[kc * P:(kc + 1) * P, :])

        masks.make_identity(nc, ident_f[:])
        nc.vector.tensor_copy(ident_bf[:], ident_f[:])
        nc.gpsimd.memset(ones_bf[:], 1.0)
        nc.vector.tensor_copy(Wb_sb[:], W_sb[:])

        # ---- pass 1 (overlapped with DMA): cast, row sumsq, t accumulation ----
        for c in range(NCH):
            x_ch = x_chs[c]
            cs = slice(c * CH, (c + 1) * CH)
            xb_ch = xb_all[:, c * CW:(c + 1) * CW]
            nc.vector.tensor_copy(xb_ch, x_ch[:])
            sq_ch = wp.tile([P, CW], BF16, tag="sq", name=f"sq{c}")
            nc.scalar.activation(sq_ch[:], x_ch[:], AF.Square)
            _rowsum(nc, nsq[:, cs], sq_ch[:].rearrange("p (t d) -> p t d", t=CH),
                    sq_ch[:].rearrange("p (t d) -> p t d", t=CH))
            nc.scalar.activation(nrm[:, cs], nsq[:, cs], AF.Sqrt)
            nc.vector.reciprocal(invn[:, cs], nrm[:, cs])
            nc.vector.tensor_copy(invn_bf[:, cs], invn[:, cs])

            if c == 0:
                # local x^T for the final GEMM - PE is idle during the load
                for i in range(TL):
                    for h in range(2):
                        pt = psW.tile([P, P], BF16, tag="pw", name=f"pt{i}_{h}")
                        nc.tensor.transpose(
                            pt[:], xb_all[:, i * D + h * P:i * D + (h + 1) * P],
                            ident_bf[:],
                        )
                        nc.vector.tensor_copy(
                            xbT[:, (2 * i + h) * P:(2 * i + h + 1) * P], pt[:]
                        )

            for i in range(c * CH, (c + 1) * CH):
                nc.tensor.matmul(
                    t_ps[:], lhsT=invn_bf[:, i:i + 1],
                    rhs=xb_all[:, i * D:(i + 1) * D],
                    start=(i == 0), stop=(i == T - 1),
                )

        # ---- barrier: t ready; broadcast to 128 partitions ----
        nc.vector.tensor_copy(t_sb[:], t_ps[:])
        nc.tensor.matmul(tb_ps[:], lhsT=ones_bf[:], rhs=t_sb[:], start=True, stop=True)
        nc.vector.tensor_copy(tb_sb[:], tb_ps[:])

        # ---- pass 2: r = x.t, f, g = f*x (Act), z accumulation (PE) ----
        tb3 = _bcast_free(tb_sb[:], CH)          # [128, CH, 256] stride-0
        for c in range(NCH):
            cs = slice(c * CH, (c + 1) * CH)
            xb3 = xb_all[:, c * CW:(c + 1) * CW].rearrange("p (t d) -> p t d", t=CH)
            u_ch = wp.tile([P, CW], BF16, tag="u", name=f"u{c}")
            u3 = u_ch[:].rearrange("p (t d) -> p t d", t=CH)
            nc.vector.tensor_mul(u3, xb3, tb3)
            _rowsum(nc, r_bf[:, cs], u3, u3)
            nc.vector.tensor_mul(p_t[:, cs], nrm[:, cs], r_bf[:, cs])
            nc.scalar.activation(sp_t[:, cs], p_t[:, cs], AF.Sqrt)
            nc.vector.reciprocal(f_t[:, cs], sp_t[:, cs])

            g_ch = wp.tile([P, CW], BF16, tag="g", name=f"g{c}")
            for j in range(CH):
                i = c * CH + j
                g_i = g_ch[:, j * D:(j + 1) * D]
                nc.scalar.mul(g_i, xb_all[:, i * D:(i + 1) * D], f_t[:, i:i + 1])
                nc.tensor.matmul(
                    z_top_ps[:], lhsT=xb_all[:, i * D:i * D + P], rhs=g_i,
                    start=(i == 0), stop=(i == T - 1),
                )
                nc.tensor.matmul(
                    z22_ps[:], lhsT=xb_all[:, i * D + P:(i + 1) * D],
                    rhs=g_ch[:, j * D + P:(j + 1) * D],
                    start=(i == 0), stop=(i == T - 1),
                )

        # ---- zw = z @ W using symmetry (z21 = z12^T) ----
        nc.vector.tensor_copy(z_top_sb[:], z_top_ps[:])
        nc.vector.tensor_copy(z22_sb[:], z22_ps[:])
        zT_ps = psW.tile([P, P], BF16, tag="pw", name="zT")
        nc.tensor.transpose(zT_ps[:], z_top_sb[:, P:D], ident_bf[:])
        nc.vector.tensor_copy(z21_sb[:], zT_ps[:])

        for half, (lhs1, lhs2) in enumerate(
            ((z_top_sb[:, 0:P], z21_sb[:]),        # zw_top = z11 W1 + (z12^T)^T W2
             (z_top_sb[:, P:D], z22_sb[:]))        # zw_bot = z12^T W1 + z22 W2
        ):
            zw_ps = psW.tile([P, D], F32, tag="pw", name=f"zw{half}")
            nc.tensor.matmul(zw_ps[:], lhsT=lhs1, rhs=Wb_sb[:, 0:D],
                             start=True, stop=False)
            nc.tensor.matmul(zw_ps[:], lhsT=lhs2, rhs=Wb_sb[:, D:2 * D],
                             start=False, stop=True)
            nc.vector.tensor_copy(zw_sb[:, half * D:(half + 1) * D], zw_ps[:])

        # ---- final: out_j = f_j * (x_j @ zw) for the 8 local tiles ----
        out3 = out.rearrange("(p j) d -> p j d", p=P)
        for i in range(TL):
            o_ps = psW.tile([P, D], F32, tag="pw", name=f"o{i}")
            for h in range(2):
                nc.tensor.matmul(
                    o_ps[:], lhsT=xbT[:, (2 * i + h) * P:(2 * i + h + 1) * P],
                    rhs=zw_sb[:, h * D:(h + 1) * D],
                    start=(h == 0), stop=(h == 1),
                )
            o_sb = wp.tile([P, D], F32, tag="osb", name=f"osb{i}")
            nc.scalar.mul(o_sb[:], o_ps[:], f_t[:, i:i + 1])
            nc.sync.dma_start(out3[:, i, :], o_sb[:])


def _build():
    nc = bacc.Bacc("TRN2", target_bir_lowering=False, debug=False, num_devices=R)
    x = nc.dram_tensor("x", [N, D], F32, kind="ExternalInput")
    W = nc.dram_tensor("W", [D, D], F32, kind="ExternalInput")
    out = nc.dram_tensor("out", [NL, D], F32, kind="ExternalOutput")
    with nc.allow_low_precision("bf16 row reductions; validated ~1.6e-3 rel err"):
        with tile.TileContext(nc) as tc:
            _program(
                tc,
                x.ap() if hasattr(x, "ap") else x,
                W.ap() if hasattr(W, "ap") else W,
                out.ap() if hasattr(out, "ap") else out,
            )
    nc.finalize()
    return nc


def _run(inputs, trace=False):
    if "nc" not in _cache:
        _cache["nc"] = _build()
    nc = _cache["nc"]
    x = np.ascontiguousarray(inputs["x"], dtype=np.float32)
    W = np.ascontiguousarray(inputs["W"], dtype=np.float32)
    in_maps = [
        {"x": np.roll(x, -r * NL, axis=0), "W": W} for r in range(R)
    ]
    res = bass_utils.run_bass_kernel_spmd(
        nc, in_maps, core_ids=list(range(R)), trace=trace,
    )
    out = np.concatenate([res.results[r]["out"] for r in range(R)], axis=0)
    return out, res


def kernel(**inputs) -> np.ndarray:
    out, _ = _run(inputs, trace=False)
    return out
